# revision 50
# baseline (speedup 1.0000x reference)
"""Trainium2 Bass kernel for EnergyConditionedEquivariantAtomAttentionLowRank.

Strategy
--------
Data-parallel over batch B=8 across 8 NeuronCores (one sample per core).
The per-(b,e,n) tensor product is never materialized: using the low-rank
structure  tp_w = base_w + (geom_c * en_c) @ rank_basis  and linearity of the
tensor product in its weights, the values tensor factors as

    values[e,n,:] = VB[n,:] + sum_r en_c[e,r] * VR[n,r,:]

where VB/VR depend only on n (per core).  The gate MLP's first layer is
factored into a per-n part (xn) and a per-e part (xe) that are broadcast-added
on chip; only the [E*N, HID] hidden activations are ever formed, in SBUF.
The gated aggregation over n becomes two small matmuls with lhsT = gate.

Weights/constants are packed host-side into a handful of DRAM tensors (one
DMA each — per-tensor loads serialized ~35us on the sync queue).  The heavy
matmul paths run in float32r (TF22), the output MLP in exact fp32.
"""

import os

import numpy as np

import concourse.bacc as bacc
import concourse.bass as bass
import concourse.tile as tile
from concourse import mybir
from concourse.alu_op_type import AluOpType
from concourse.bass_utils import run_bass_kernel_spmd

dt = mybir.dt
AF = mybir.ActivationFunctionType
F32 = dt.float32
F32R = dt.float32r

B, N, NE, DE = 8, 64, 128, 16
MUL = 16
RBF = 32
ZEMB = 32
HID = 128
LAT = 64
RANK = 8
CUTOFF = 6.0
NZ = 101  # MAX_Z + 1
SQ3 = float(np.sqrt(3.0))
C0 = float(1.0 / np.sqrt(2.0 * MUL))
C3 = float(C0 / np.sqrt(3.0))
_DELTA = CUTOFF / (RBF - 1)
GAMMA = float(1.0 / (_DELTA * _DELTA + 1e-12))
PI = float(np.pi)

NB = 8                 # n-values per chunk of the gate loop
NCH = N // NB          # 8 chunks
CW = NB * NE           # 1024 columns per chunk

# True -> run everything in plain fp32 (slower, for accuracy comparison)
FP32_ALL = os.environ.get("FP32_ALL", "0") == "1"
# gate MLP layers 2/3 dtype: f16 (fast weight load) or f32r
GATE16 = os.environ.get("GATE16", "1") == "1" and not FP32_ALL
F16 = dt.float16
# CoreSim does not implement the fused Silu activation; set False to emit
# sigmoid+mul instead (slower, simulation-only)
SILU_FUSED = True
DEBUG = False
_dbg_shapes = {}


def _rdt():
    return F32 if FP32_ALL else F32R


# ---------------------------------------------------------------------------
# pack layouts: name -> (part_rows, col_off, width) within each pack
# ---------------------------------------------------------------------------
PA = {  # f32 [128, *]
    "wb2": (128, 0, 128), "wc2": (128, 128, 128), "we2": (128, 256, 128),
    "wo2": (128, 384, 128), "ident": (128, 512, 128),
    "wc3": (128, 640, 8), "we3": (128, 648, 8), "wo3": (128, 656, 64),
    "bb1": (128, 720, 1), "bb2": (128, 721, 1), "bc1": (128, 722, 1),
    "bc2": (128, 723, 1), "be1": (128, 724, 1), "be2": (128, 725, 1),
    "bo1": (128, 726, 1), "bo2": (128, 727, 1), "bs1": (128, 728, 1),
    "bs2": (128, 729, 1), "bs3": (128, 730, 1), "bc3": (8, 731, 1),
    "be3": (8, 732, 1),
}
PA_W = 736
PR = {  # f32r [128, *]
    "ws2": (128, 0, 128), "wb3": (128, 128, 1024), "ws3sc": (128, 1152, 15),
}
PR_W = 1168
PB = {  # f32 [64, *]
    "wb1": (64, 0, 128), "wc1": (64, 128, 128), "ws1zr": (64, 256, 128),
    "cent": (64, 384, 32),
}
PB_W = 416
PD = {  # f32 [32, *]
    "ws1a": (32, 0, 128), "ws1n": (32, 128, 128), "wo1": (32, 256, 128),
}
PD_W = 384
PE16 = {  # f32 [16, *]
    "eft": (16, 0, 128), "we1": (16, 128, 128), "ws1e": (16, 256, 128),
}
PE16_W = 384
PER = {  # f32r [16, *]
    "rba": (16, 0, 128), "rbb": (16, 128, 128), "rbc": (16, 256, 128),
    "rbd": (16, 384, 128),
}
PER_W = 512
PF = {  # f32 [1, *]
    "bo3": (1, 0, 64), "ones": (1, 64, 128),
}
PF_W = 192
PFR = {  # f32r [1, *]
    "bb3": (1, 0, 1024), "onesr": (1, 1024, 128),
}
PFR_W = 1152
PH = {  # gate dtype [128, *]
    "ws2", "ws3sc",
}
PCORE = {  # f32 [64, *]  (per core)
    "h": (64, 0, 64), "posp": (64, 64, 3), "maskf": (64, 67, 1),
}
PCORE_W = 68

PACKS = [
    ("pA", PA, PA_W, 128, False), ("pR", PR, PR_W, 128, True),
    ("pB", PB, PB_W, 64, False), ("pD", PD, PD_W, 32, False),
    ("pE", PE16, PE16_W, 16, False), ("pEr", PER, PER_W, 16, True),
    ("pF", PF, PF_W, 1, False), ("pFr", PFR, PFR_W, 1, True),
    ("pC", PCORE, PCORE_W, 64, False),
]
PHG = {  # gate weights [128, *] in the gate dtype (f16 or f32r)
    "ws2g": (128, 0, 128), "ws3g": (128, 128, 15),
}
PHG_W = 144


def build_module():
    nc = bacc.Bacc("TRN2", target_bir_lowering=False, debug=False, num_devices=8)

    d = {}
    for pname, layout, width, rows, is_r in PACKS:
        dty = _rdt() if is_r else F32
        d[pname] = nc.dram_tensor(pname, [rows, width], dty,
                                  kind="ExternalInput").ap()
    gdt = F16 if GATE16 else _rdt()
    d["pH"] = nc.dram_tensor("pH", [128, PHG_W], gdt,
                             kind="ExternalInput").ap()
    d["ohz"] = nc.dram_tensor("ohz", [NZ, N], F32, kind="ExternalInput").ap()
    d["zemb"] = nc.dram_tensor("zemb", [NZ, ZEMB], F32,
                               kind="ExternalInput").ap()

    out_d = nc.dram_tensor("out", [NE, LAT], F32, kind="ExternalOutput").ap()

    with tile.TileContext(nc) as tc:
        _emit(nc, tc, d, out_d)

    nc.compile()
    return nc, list(d.keys())


def _emit(nc, tc, d, out_d):
    from contextlib import ExitStack
    ctx = ExitStack()
    with ctx:
        singles = ctx.enter_context(tc.tile_pool(name="singles", bufs=1))
        work = ctx.enter_context(tc.tile_pool(name="work", bufs=2))
        big = ctx.enter_context(tc.tile_pool(name="big", bufs=2))
        pbig = ctx.enter_context(tc.tile_pool(name="pbig", bufs=2, space="PSUM"))
        psc = ctx.enter_context(tc.tile_pool(name="psc", bufs=2, space="PSUM"))
        psm = ctx.enter_context(tc.tile_pool(name="psm", bufs=2, space="PSUM"))

        dma = nc.sync.dma_start
        vec = nc.vector
        act = nc.scalar
        RDT = _rdt()

        def silu(out, in_, bias=0.0):
            if SILU_FUSED:
                act.activation(out=out, in_=in_, func=AF.Silu, bias=bias)
            else:
                z = work.tile(list(out.shape), F32, tag="silu_z")
                act.activation(out=z[:], in_=in_, func=AF.Identity, bias=bias)
                act.activation(out=out, in_=z[:], func=AF.Sigmoid)
                vec.tensor_mul(out, out, z[:])

        def dbg(name, t):
            if DEBUG:
                if t.dtype == F32R:
                    t = t.bitcast(F32)
                elif t.dtype == F16:
                    return
                shape = list(t.shape)
                _dbg_shapes[name] = shape
                dd = nc.dram_tensor("dbg_" + name, shape, F32,
                                    kind="ExternalOutput").ap()
                dma(out=dd, in_=t)

        # ---------- load the packs (one DMA each) ----------
        packs = {}
        W = {}
        for pname, layout, width, rows, is_r in PACKS:
            ap = d[pname]
            t = singles.tile([rows, width], ap.dtype, tag=pname)
            dma(out=t[:], in_=ap)
            packs[pname] = t
            for wname, (r, off, wdt) in layout.items():
                W[wname] = t[0:r, off:off + wdt]
        pH = singles.tile([128, PHG_W], d["pH"].dtype, tag="pH")
        dma(out=pH[:], in_=d["pH"])
        for wname, (r_, off_, wdt_) in PHG.items():
            W[wname] = pH[0:r_, off_:off_ + wdt_]
        ohz = singles.tile([NZ, N], F32, tag="ohz")
        dma(out=ohz[:], in_=d["ohz"])
        zemb = singles.tile([NZ, ZEMB], F32, tag="zemb")
        dma(out=zemb[:], in_=d["zemb"])

        h = W["h"]
        posp = W["posp"]
        maskf = W["maskf"]
        cent = W["cent"]
        ident = W["ident"]
        eft = W["eft"]

        # broadcast absorber position to all rows (DMA from DRAM, 0-stride)
        pos0 = singles.tile([N, 3], F32, tag="pos0")
        _, poff, _ = PCORE["posp"]
        dma(out=pos0[:], in_=bass.AP(
            tensor=d["pC"].tensor, offset=poff,
            ap=[[0, N], [1, 3]]))

        # small bias constants (const_aps only pre-registers 0.0 / 1.0)
        eps8 = singles.tile([128, 1], F32, tag="eps8")
        vec.memset(eps8[:], 1e-8)
        halfpi = singles.tile([N, 1], F32, tag="halfpi")
        vec.memset(halfpi[:], PI / 2.0)

        # ---------- stage A: geometry (N on partitions) ----------
        rel = singles.tile([N, 3], F32, tag="rel")
        vec.tensor_sub(rel[:], posp, pos0[:])
        sq = singles.tile([N, 3], F32, tag="sq")
        vec.tensor_mul(sq[:], rel[:], rel[:])
        r2 = singles.tile([N, 1], F32, tag="r2")
        vec.reduce_sum(out=r2[:], in_=sq[:], axis=mybir.AxisListType.X)
        r = singles.tile([N, 1], F32, tag="r")
        act.activation(out=r[:], in_=r2[:], func=AF.Sqrt)

        # invariants sqrt right after r sqrt (shared ACT table)
        hv = h[:, MUL:64].rearrange("p (u i) -> p u i", i=3)
        vsq = singles.tile([N, 48], F32, tag="vsq")
        vec.tensor_mul(vsq.rearrange("p (u i) -> p u i", i=3), hv, hv)
        msum = singles.tile([N, MUL], F32, tag="msum")
        vec.reduce_sum(out=msum[:], in_=vsq.rearrange("p (u i) -> p u i", i=3),
                       axis=mybir.AxisListType.X)
        inv = singles.tile([N, 32], F32, tag="inv")
        vec.tensor_copy(out=inv[:, 0:MUL], in_=h[:, 0:MUL])
        act.activation(out=inv[:, MUL:32], in_=msum[:], func=AF.Sqrt,
                       scale=1.0 / 3.0, bias=eps8[:N])

        rc = singles.tile([N, 1], F32, tag="rc")
        vec.tensor_scalar_min(out=rc[:], in0=r[:], scalar1=CUTOFF)
        rmx = singles.tile([N, 1], F32, tag="rmx")
        vec.tensor_scalar_max(out=rmx[:], in0=r[:], scalar1=1e-8)
        rinv = singles.tile([N, 1], F32, tag="rinv")
        vec.reciprocal(out=rinv[:], in_=rmx[:])

        # sh1 = sqrt(3) * u  (pos columns pre-permuted on host)
        sh1 = singles.tile([N, 3], F32, tag="sh1")
        vec.tensor_scalar(out=sh1[:], in0=rel[:], scalar1=rinv[:, 0:1],
                          scalar2=SQ3, op0=AluOpType.mult, op1=AluOpType.mult)

        # RBF: rr = exp(-gamma * (cent - rc)^2)
        dgs = singles.tile([N, RBF], F32, tag="dgs")
        vec.tensor_scalar_sub(out=dgs[:], in0=cent, scalar1=rc[:, 0:1])
        vec.tensor_mul(dgs[:], dgs[:], dgs[:])
        rr = singles.tile([N, RBF], F32, tag="rr")
        act.activation(out=rr[:], in_=dgs[:], func=AF.Exp, scale=-GAMMA)

        # cwv = 0.5*(cos(pi*rc/6)+1) * (r<=6) * mask, absorber row zeroed
        # cos(x) = sin(pi/2 - x), x = pi*rc/6 in [0, pi]
        cosr = singles.tile([N, 1], F32, tag="cosr")
        act.activation(out=cosr[:], in_=rc[:], func=AF.Sin,
                       scale=-PI / CUTOFF, bias=halfpi[:])
        vcut = singles.tile([N, 1], F32, tag="vcut")
        vec.tensor_scalar(out=vcut[:], in0=r[:], scalar1=CUTOFF, scalar2=None,
                          op0=AluOpType.is_le)
        cwv = singles.tile([N, 1], F32, tag="cwv")
        vec.tensor_scalar(out=cwv[:], in0=cosr[:], scalar1=0.5, scalar2=0.5,
                          op0=AluOpType.mult, op1=AluOpType.add)
        vec.tensor_mul(cwv[:], cwv[:], vcut[:])
        vec.tensor_mul(cwv[:], cwv[:], maskf)
        vec.memset(cwv[0:1, :], 0.0)
        dbg('cwv', cwv[:])

        # vs1[n,u] = sum_i v1[n,u,i]*sh1[n,i]
        vtmp = singles.tile([N, 48], F32, tag="vtmp")
        sh1b = sh1.unsqueeze(1).broadcast_to([N, MUL, 3])
        vec.tensor_mul(vtmp.rearrange("p (u i) -> p u i", i=3), hv, sh1b)
        vs1 = singles.tile([N, MUL], F32, tag="vs1")
        vec.reduce_sum(out=vs1[:], in_=vtmp.rearrange("p (u i) -> p u i", i=3),
                       axis=mybir.AxisListType.X)
        dbg('vs1', vs1[:])
        dbg('sh1', sh1[:])
        dbg('inv', inv[:])

        # ---------- transposes (feat on partitions) ----------
        def transpose_to(src_ap, p_out, f_out, tag, dty=F32):
            ps = psm.tile([p_out, f_out], F32, tag="ps")
            nc.tensor.transpose(out=ps[:], in_=src_ap,
                                identity=ident[:f_out, :f_out])
            t = singles.tile([p_out, f_out], dty, tag=tag)
            vec.tensor_copy(out=t[:], in_=ps[:])
            return t

        invT = transpose_to(inv[:], 32, N, "invT")
        s1T = transpose_to(h[:, 0:MUL], MUL, N, "s1T", RDT)
        vs1T = transpose_to(vs1[:], MUL, N, "vs1T", RDT)
        vT = [transpose_to(h[:, MUL + i:64:3], MUL, N, f"vT{i}", RDT)
              for i in range(3)]

        # zr in n-major layout via one-hot matmul: ohz.T @ zemb -> [N, 32]
        ps_zr = psm.tile([N, 32], F32, tag="ps")
        nc.tensor.matmul(ps_zr[:], ohz[:], zemb[:], start=True, stop=True)
        # geom_nm = [zr | rr] (n on partitions), then one transpose -> [64, N]
        geom_nm = singles.tile([N, 64], F32, tag="geom_nm")
        vec.tensor_copy(out=geom_nm[:, 0:32], in_=ps_zr[:])
        vec.tensor_copy(out=geom_nm[:, 32:64], in_=rr[:])
        geomT = transpose_to(geom_nm[:], 64, N, "geomT")
        dbg('geom_nm', geom_nm[:])

        # ---------- stage B: small MLPs ----------
        def mlp2(w1, b1, w2, b2, rhs, cols, tag, out_dt=F32):
            ps = psm.tile([HID, cols], F32, tag="ps")
            nc.tensor.matmul(ps[:], w1, rhs, start=True, stop=True)
            h1 = work.tile([HID, cols], F32, tag="h1_" + tag)
            silu(h1[:], ps[:], bias=b1)
            ps2 = psm.tile([HID, cols], F32, tag="ps")
            nc.tensor.matmul(ps2[:], w2, h1[:], start=True, stop=True)
            h2 = singles.tile([HID, cols], out_dt, tag="h2_" + tag)
            silu(h2[:], ps2[:], bias=b2)
            return h2

        # base_w trunk on geom_in = [zr, rr]
        h2b = mlp2(W["wb1"], W["bb1"], W["wb2"], W["bb2"], geomT[:], N, "b",
                   out_dt=RDT)

        # base_w[n, :] = h2b.T @ wb3 + bb3   (n on partitions; bias via K=1)
        bw = singles.tile([N, 1024], F32, tag="bw")
        for j in range(2):
            ps = psm.tile([N, 512], F32, tag="ps")
            nc.tensor.matmul(ps[:], h2b[:], W["wb3"][:, 512 * j:512 * (j + 1)],
                             start=True, stop=False)
            nc.tensor.matmul(ps[:], W["onesr"][:, 0:N],
                             W["bb3"][:, 512 * j:512 * (j + 1)],
                             start=False, stop=True)
            vec.tensor_copy(out=bw[:, 512 * j:512 * (j + 1)], in_=ps[:])

        # geom_c
        h2c = mlp2(W["wc1"], W["bc1"], W["wc2"], W["bc2"], geomT[:], N, "c")
        ps_gc = psm.tile([RANK, N], F32, tag="ps")
        nc.tensor.matmul(ps_gc[:], W["wc3"], h2c[:], start=True, stop=True)
        gcT = singles.tile([RANK, N], F32, tag="gcT")
        vec.tensor_scalar_add(out=gcT[:], in0=ps_gc[:], scalar1=W["bc3"])
        gc = transpose_to(gcT[:], N, RANK, "gc")
        dbg('bw', bw[:])
        dbg('gc', gc[:])

        # energy_c (E rows)
        h2e = mlp2(W["we1"], W["be1"], W["we2"], W["be2"], eft, NE, "e")
        ps_ec = psm.tile([RANK, NE], F32, tag="ps")
        nc.tensor.matmul(ps_ec[:], W["we3"], h2e[:], start=True, stop=True)
        ecT = singles.tile([RANK, NE], F32, tag="ecT")
        vec.tensor_scalar_add(out=ecT[:], in0=ps_ec[:], scalar1=W["be3"])
        ec = transpose_to(ecT[:], NE, RANK, "ec")
        dbg('ec', ec[:])

        # ---------- gate layer-1 factored parts ----------
        # xe[k, e] = ws1e.T @ e_feat.T + (bs1 + ws1a.T @ inv_abs)  broadcast
        ps_b1 = psm.tile([HID, 1], F32, tag="ps")
        nc.tensor.matmul(ps_b1[:], W["ws1a"], invT[:, 0:1], start=True,
                         stop=True)
        b1eff = singles.tile([HID, 1], F32, tag="b1eff")
        vec.tensor_add(b1eff[:], ps_b1[:], W["bs1"])
        ps_xe = psm.tile([HID, NE], F32, tag="ps")
        nc.tensor.matmul(ps_xe[:], W["ws1e"], eft, start=True, stop=True)
        xe = singles.tile([HID, NE], F32, tag="xe")
        vec.tensor_scalar_add(out=xe[:], in0=ps_xe[:], scalar1=b1eff[:, 0:1])
        dbg('xeT', xe[:])

        # xn[k, n] = ws1n.T@inv + [ws1z;ws1r].T@[zr;rr]
        ps_xn = psm.tile([HID, N], F32, tag="ps")
        nc.tensor.matmul(ps_xn[:], W["ws1n"], invT[:], start=True, stop=False)
        nc.tensor.matmul(ps_xn[:], W["ws1zr"], geomT[:], start=False, stop=True)
        xn = singles.tile([HID, N], F32, tag="xn")
        vec.tensor_copy(out=xn[:], in_=ps_xn[:])
        dbg('xnT', xn[:])

        # ---------- stage D: VB / VR (values factorization) ----------
        # VRB[n, 0:512]   = VR  (r outer, d inner; includes geom_c factor)
        # VRB[n, 512:576] = VB
        # VRB[n, 576]     = 1.0  (gate-sum column)
        VRB = singles.tile([N, 578], RDT, tag="VRB")
        VR = VRB[:, 0:512].rearrange("p (r e) -> p r e", r=RANK)
        VB = VRB[:, 512:576]
        act.activation(out=VRB[:, 576:578],
                       in_=eps8[:N].broadcast_to([N, 2]), func=AF.Copy,
                       bias=1.0, scale=0.0)

        # --- base paths from bw (layout: path p, u, w at 256p+16u+w) ---
        sv1 = singles.tile([N, 2 * MUL], F32, tag="sv1")  # [s1 | vs1]
        vec.tensor_copy(out=sv1[:, 0:MUL], in_=h[:, 0:MUL])
        vec.tensor_copy(out=sv1[:, MUL:2 * MUL], in_=vs1[:])
        tAD = singles.tile([N, 512], F32, tag="tAD")
        bwAD = bass.AP(tensor=bw.tensor, offset=bw.offset,
                       ap=[list(bw.ap[0]), [768, 2], [16, MUL], [1, MUL]])
        sv1b = sv1.rearrange("p (g u) -> p g u", g=2).unsqueeze(3) \
                  .broadcast_to([N, 2, MUL, MUL])
        vec.tensor_mul(tAD.rearrange("p (g u w) -> p g u w", g=2, u=MUL),
                       bwAD, sv1b)
        sAD = singles.tile([N, 2 * MUL], F32, tag="sAD")
        vec.reduce_sum(out=sAD[:],
                       in_=tAD.rearrange("p (g u w) -> p g w u", g=2, u=MUL),
                       axis=mybir.AxisListType.X)
        vec.tensor_add(VB[:, 0:MUL], sAD[:, 0:MUL], sAD[:, MUL:2 * MUL])

        # B path: Bs[n,w] = sum_u bwB[u,w]*s1[u]
        tB = singles.tile([N, 256], F32, tag="tB")
        s1b = h[:, 0:MUL].unsqueeze(2).broadcast_to([N, MUL, MUL])
        vec.tensor_mul(tB.rearrange("p (u w) -> p u w", u=MUL),
                       bw[:, 256:512].rearrange("p (u w) -> p u w", u=MUL), s1b)
        Bs = singles.tile([N, MUL], F32, tag="Bs")
        vec.reduce_sum(out=Bs[:], in_=tB.rearrange("p (u w) -> p w u", u=MUL),
                       axis=mybir.AxisListType.X)

        # C path: Cv[n,w,i] = sum_u bwC[u,w]*v1[n,u,i]
        tC = singles.tile([N, 768], F32, tag="tC")
        bwC = bw[:, 512:768].rearrange("p (u w) -> p u w", u=MUL).unsqueeze(3) \
                            .broadcast_to([N, MUL, MUL, 3])
        hvb = h[:, MUL:64].rearrange("p (u i) -> p u i", i=3).unsqueeze(2) \
                          .broadcast_to([N, MUL, MUL, 3])
        vec.tensor_mul(tC.rearrange("p (u w i) -> p u w i", u=MUL, w=MUL),
                       bwC, hvb)
        Cv = singles.tile([N, 48], F32, tag="Cv")
        vec.reduce_sum(out=Cv[:],
                       in_=tC.rearrange("p (u w i) -> p w i u", u=MUL, w=MUL),
                       axis=mybir.AxisListType.X)

        # VB vector part: Bs[w]*sh1[i] + Cv[w,i]
        Bsb = Bs.unsqueeze(2).broadcast_to([N, MUL, 3])
        sh1b2 = sh1.unsqueeze(1).broadcast_to([N, MUL, 3])
        tBv = singles.tile([N, 48], F32, tag="tBv")
        vec.tensor_mul(tBv.rearrange("p (w i) -> p w i", i=3), Bsb, sh1b2)
        vec.tensor_add(VB[:, MUL:64], tBv[:], Cv[:])

        # --- rank paths (matmuls, K=16, f32r) ---
        ps_r1 = psm.tile([N, 384], F32, tag="ps")  # [sA | sB | sD]
        nc.tensor.matmul(ps_r1[:, 0:128], s1T[:], W["rba"], start=True,
                         stop=True)
        nc.tensor.matmul(ps_r1[:, 128:256], s1T[:], W["rbb"], start=True,
                         stop=True)
        nc.tensor.matmul(ps_r1[:, 256:384], vs1T[:], W["rbd"], start=True,
                         stop=True)
        ps_r2 = psm.tile([N, 384], F32, tag="ps")  # [PC0 | PC1 | PC2]
        for i in range(3):
            nc.tensor.matmul(ps_r2[:, 128 * i:128 * (i + 1)], vT[i][:],
                             W["rbc"], start=True, stop=True)

        # move to SBUF (DVE can read only one PSUM input per op)
        r1 = singles.tile([N, 384], F32, tag="r1")
        vec.tensor_copy(out=r1[:], in_=ps_r1[:])
        # VR scalar part: gc[r] * (sA + sD)   (layout [r, w] at 16r+w)
        tS = singles.tile([N, 128], F32, tag="tS")
        vec.tensor_add(tS[:], r1[:, 0:128], r1[:, 256:384])
        gcb = gc.unsqueeze(2).broadcast_to([N, RANK, MUL])
        vec.tensor_mul(VR[:, :, 0:MUL].rearrange("p r w -> p r w"),
                       tS.rearrange("p (r w) -> p r w", r=RANK), gcb)

        # VR vector part: gc[r] * (sB[r,w]*sh1[i] + PC_i[r,w])
        for i in range(3):
            tV = singles.tile([N, 128], F32, tag="tV")
            nc.vector.scalar_tensor_tensor(
                out=tV[:], in0=r1[:, 128:256], scalar=sh1[:, i:i + 1],
                in1=ps_r2[:, 128 * i:128 * (i + 1)],
                op0=AluOpType.mult, op1=AluOpType.add)
            vec.tensor_mul(
                VR[:, :, MUL + i:64:3].rearrange("p r w -> p r w"),
                tV.rearrange("p (r w) -> p r w", r=RANK), gcb)

        dbg('VRB', VRB[:])

        # ---------- stage C: gate MLP over E*N rows (n-outer chunks) ----------
        # scores accumulate into [8, 512] psum tiles: the matmul for score
        # half-chunk s uses a shifted zero-padded ws3 (col s%8 = ws3) so its
        # 512 scores land in psum row s%8, zeros elsewhere.
        g_nm = singles.tile([N, NE], RDT, tag="g_nm")  # sigmoid(score)*cwv
        ps_sc = None
        sc_tiles = []
        for c in range(NCH):
            h1p = big.tile([HID, CW], F32, tag="h1p")
            xeb = xe.unsqueeze(1).broadcast_to([HID, NB, NE])
            xnb = xn[:, NB * c:NB * (c + 1)].unsqueeze(2) \
                    .broadcast_to([HID, NB, NE])
            vec.tensor_tensor(h1p.rearrange("p (n e) -> p n e", n=NB), xeb,
                              xnb, op=AluOpType.add)
            h1 = big.tile([HID, CW], F16 if GATE16 else RDT, tag="h1")
            silu(h1[:], h1p[:])
            if c == 0:
                dbg('h1c0', h1[:])
            z2 = pbig.tile([HID, CW], F32, tag="z2")
            for j in range(2):
                sl = slice(512 * j, 512 * (j + 1))
                nc.tensor.matmul(z2[:, sl], W["ws2g"], h1[:, sl],
                                 start=True, stop=True)
            h2 = big.tile([HID, CW], F16 if GATE16 else RDT, tag="h2")
            silu(h2[:], z2[:], bias=W["bs2"])
            if c == 0:
                dbg('h2c0', h2[:])
            for j in range(2):
                s = 2 * c + j          # score half-chunk: n in [4s, 4s+4)
                q = s % 8
                if q == 0:
                    ps_sc = psc.tile([8, 512], F32, tag="ps_sc")
                    sc_tiles.append(ps_sc)
                sl = slice(512 * j, 512 * (j + 1))
                nc.tensor.matmul(ps_sc[:], W["ws3g"][:, 7 - q:15 - q],
                                 h2[:, sl], start=(q == 0), stop=(q == 7))
        # sigmoids grouped at the end (one ACT table load)
        for t, pst in enumerate(sc_tiles):
            sig = work.tile([8, 512], RDT, tag="sig")
            act.activation(out=sig[:], in_=pst[:], func=AF.Sigmoid,
                           bias=W["bs3"][:8])
            if t == 0:
                dbg('sig0', sig[:])
            srcv = sig[:].rearrange("p (n e) -> p n e", n=4)
            dma(out=g_nm[32 * t:32 * (t + 1), :], in_=srcv)

        # gate = sigmoid(score) * cwv
        vec.tensor_scalar_mul(out=g_nm[:], in0=g_nm[:].bitcast(F32),
                              scalar1=cwv[:, 0:1])
        dbg('g_nm', g_nm[:])

        # ---------- aggregation: G = g_nm.T @ VRB ----------
        ps_G = pbig.tile([NE, 578], F32, tag="z2")
        for sl in [slice(0, 512), slice(512, 578)]:
            nc.tensor.matmul(ps_G[:, sl], g_nm[:, :], VRB[:, sl],
                             start=True, stop=True)

        # agg = (G1 + sum_r ec[e,r]*G2[e,r,:]) / max(gsum, 1e-8)
        gs = singles.tile([NE, 1], F32, tag="gs")
        vec.tensor_scalar_max(out=gs[:], in0=ps_G[:, 576:577], scalar1=1e-8)
        gsi = singles.tile([NE, 1], F32, tag="gsi")
        vec.reciprocal(out=gsi[:], in_=gs[:])

        G2v = ps_G[:, 0:512].rearrange("p (r e) -> p e r", r=RANK)
        ecb = ec.unsqueeze(1).broadcast_to([NE, 64, RANK])
        tG = singles.tile([NE, 512], F32, tag="tG")
        vec.tensor_mul(tG.rearrange("p (e r) -> p e r", r=RANK), G2v, ecb)
        aggR = singles.tile([NE, 64], F32, tag="aggR")
        vec.reduce_sum(out=aggR[:],
                       in_=tG.rearrange("p (e r) -> p e r", r=RANK),
                       axis=mybir.AxisListType.X)
        agg = singles.tile([NE, 64], F32, tag="agg")
        vec.tensor_add(agg[:], aggR[:], ps_G[:, 512:576])
        vec.tensor_scalar_mul(out=agg[:], in0=agg[:], scalar1=gsi[:, 0:1])
        dbg('agg', agg[:])

        # ---------- invariants(agg) + out MLP (exact fp32) ----------
        av = agg[:, MUL:64].rearrange("p (u i) -> p u i", i=3)
        asq = singles.tile([NE, 48], F32, tag="asq")
        vec.tensor_mul(asq.rearrange("p (u i) -> p u i", i=3), av, av)
        ams = singles.tile([NE, MUL], F32, tag="ams")
        vec.reduce_sum(out=ams[:],
                       in_=asq.rearrange("p (u i) -> p u i", i=3),
                       axis=mybir.AxisListType.X)
        ia = singles.tile([NE, 32], F32, tag="ia")
        vec.tensor_copy(out=ia[:, 0:MUL], in_=agg[:, 0:MUL])
        act.activation(out=ia[:, MUL:32], in_=ams[:], func=AF.Sqrt,
                       scale=1.0 / 3.0, bias=eps8[:])
        iaT = transpose_to(ia[:], 32, NE, "iaT")

        h2o = mlp2(W["wo1"], W["bo1"], W["wo2"], W["bo2"], iaT[:], NE, "o")

        ps_out = psm.tile([NE, LAT], F32, tag="ps")
        nc.tensor.matmul(ps_out[:], h2o[:], W["wo3"], start=True, stop=False)
        nc.tensor.matmul(ps_out[:], W["ones"][:, 0:NE], W["bo3"],
                         start=False, stop=True)
        osb = singles.tile([NE, LAT], F32, tag="osb")
        vec.tensor_copy(out=osb[:], in_=ps_out[:])
        dma(out=out_d, in_=osb[:])


# ---------------------------------------------------------------------------
# host side
# ---------------------------------------------------------------------------
_CACHED = {}


def _round_fp22(x):
    """round fp32 to nearest-even at 13 mantissa bits (FP22 / fp32r)"""
    if FP32_ALL:
        return np.asarray(x, np.float32)
    u = np.ascontiguousarray(np.asarray(x, np.float32)).view(np.uint32)
    lsb = (u >> 13) & 1
    u = (u + 0x0FFF + lsb) & np.uint32(0xFFFFE000)
    return u.view(np.float32)


def _prep_maps(h_full, z, pos, mask, e_feat, absorber_index, z_emb,
               base_w_mlp, geom_c_mlp, energy_c_mlp, score_mlp, out_mlp,
               rank_basis):
    f = np.float32
    a = int(absorber_index)
    h_full = np.asarray(h_full, f)
    z = np.asarray(z)
    pos = np.asarray(pos, f)
    mask = np.asarray(mask)
    if a != 0:
        h_full = np.roll(h_full, -a, axis=1)
        z = np.roll(z, -a, axis=1)
        pos = np.roll(pos, -a, axis=1)
        mask = np.roll(mask, -a, axis=1)

    e_feat = np.asarray(e_feat, f)
    z_emb = np.asarray(z_emb, f)
    rank_basis = np.asarray(rank_basis, f)

    def p(t):
        return np.ascontiguousarray(np.asarray(t, f))

    vals = {}
    (Wb1, bb1), (Wb2, bb2), (Wb3, bb3) = base_w_mlp
    scale = np.concatenate([np.full(256, s, f) for s in (C0, C0, C0, C3)])
    vals["wb1"] = p(Wb1); vals["bb1"] = p(bb1)[:, None]
    vals["wb2"] = p(Wb2); vals["bb2"] = p(bb2)[:, None]
    vals["wb3"] = p(Wb3) * scale
    vals["bb3"] = (p(bb3) * scale)[None, :]

    (Wc1, bc1), (Wc2, bc2), (Wc3, bc3) = geom_c_mlp
    vals["wc1"] = p(Wc1); vals["bc1"] = p(bc1)[:, None]
    vals["wc2"] = p(Wc2); vals["bc2"] = p(bc2)[:, None]
    vals["wc3"] = p(Wc3); vals["bc3"] = p(bc3)[:, None]

    (We1, be1), (We2, be2), (We3, be3) = energy_c_mlp
    vals["we1"] = p(We1); vals["be1"] = p(be1)[:, None]
    vals["we2"] = p(We2); vals["be2"] = p(be2)[:, None]
    vals["we3"] = p(We3); vals["be3"] = p(be3)[:, None]

    (Ws1, bs1), (Ws2, bs2), (Ws3, bs3) = score_mlp
    Ws1 = p(Ws1)
    vals["ws1a"] = Ws1[0:32]
    vals["ws1n"] = Ws1[32:64]
    vals["ws1zr"] = Ws1[64:128]
    vals["ws1e"] = Ws1[128:144]
    vals["bs1"] = p(bs1)[:, None]
    vals["ws2"] = p(Ws2)
    vals["bs2"] = p(bs2)[:, None]
    w3p = np.zeros((HID, 15), f)
    w3p[:, 7] = p(Ws3)[:, 0]
    vals["ws3sc"] = w3p
    vals["bs3"] = np.full((128, 1), float(np.asarray(bs3, f)[0]), f)

    (Wo1, bo1), (Wo2, bo2), (Wo3, bo3) = out_mlp
    vals["wo1"] = p(Wo1); vals["bo1"] = p(bo1)[:, None]
    vals["wo2"] = p(Wo2); vals["bo2"] = p(bo2)[:, None]
    vals["wo3"] = p(Wo3); vals["bo3"] = p(bo3)[None, :]

    rb = rank_basis.reshape(RANK, 4, MUL, MUL)
    for name, pi, s in [("rba", 0, C0), ("rbb", 1, C0), ("rbc", 2, C0),
                        ("rbd", 3, C3)]:
        vals[name] = np.ascontiguousarray(
            (rb[:, pi] * s).transpose(1, 0, 2).reshape(MUL, RANK * MUL))

    centers = np.linspace(0.0, CUTOFF, RBF, dtype=f)
    vals["cent"] = np.ascontiguousarray(np.broadcast_to(centers, (N, RBF)))
    vals["ident"] = np.eye(128, dtype=f)
    vals["ones"] = np.ones((1, 128), f)
    vals["onesr"] = np.ones((1, 128), f)
    vals["eft"] = p(e_feat.T)

    def build_pack(layout, width, rows, is_r):
        buf = np.zeros((rows, width), f)
        for wname, (r, off, wdt) in layout.items():
            if wname in ("h", "posp", "maskf"):
                continue
            v = vals[wname]
            assert v.shape == (r, wdt), (wname, v.shape, (r, wdt))
            buf[0:r, off:off + wdt] = v
        return _round_fp22(buf) if is_r else buf

    shared = {}
    for pname, layout, width, rows, is_r in PACKS:
        if pname == "pC":
            continue
        shared[pname] = build_pack(layout, width, rows, is_r)
    ph = np.zeros((128, PHG_W), np.float32)
    ph[:, 0:128] = vals["ws2"]
    ph[:, 128:143] = vals["ws3sc"]
    shared["pH"] = ph.astype(np.float16) if GATE16 else _round_fp22(ph)
    shared["zemb"] = p(z_emb)

    maps = []
    for b in range(B):
        m = dict(shared)
        pc = np.zeros((64, PCORE_W), f)
        pc[:, 0:64] = h_full[b]
        pc[:, 64:67] = pos[b][:, [1, 2, 0]]
        pc[:, 67:68] = np.asarray(mask[b], f)[:, None]
        m["pC"] = pc
        oh = np.zeros((NZ, N), f)
        oh[np.asarray(z[b], np.int64), np.arange(N)] = 1.0
        m["ohz"] = oh
        maps.append(m)
    return maps


def kernel(**inputs) -> np.ndarray:
    if "nc" not in _CACHED:
        nc, in_names = build_module()
        _CACHED["nc"] = nc
        _CACHED["in_names"] = in_names
    nc = _CACHED["nc"]
    maps = _prep_maps(**inputs)
    res = run_bass_kernel_spmd(nc, maps, list(range(B)))
    return np.stack([res.results[b]["out"] for b in range(B)], axis=0)


if __name__ == "__main__":
    nc, names = build_module()
    print("built ok", len(names), "inputs")


# revision 51
# speedup vs baseline: 1.0541x; 1.0541x over previous
"""Trainium2 Bass kernel for EnergyConditionedEquivariantAtomAttentionLowRank.

Strategy
--------
Data-parallel over batch B=8 across 8 NeuronCores (one sample per core).
The per-(b,e,n) tensor product is never materialized: using the low-rank
structure  tp_w = base_w + (geom_c * en_c) @ rank_basis  and linearity of the
tensor product in its weights, the values tensor factors as

    values[e,n,:] = VB[n,:] + sum_r en_c[e,r] * VR[n,r,:]

where VB/VR depend only on n (per core).  The gate MLP's first layer is
factored into a per-n part (xn) and a per-e part (xe) that are broadcast-added
on chip; only the [E*N, HID] hidden activations are ever formed, in SBUF.
The gated aggregation over n becomes two small matmuls with lhsT = gate.

Weights/constants are packed host-side into a handful of DRAM tensors (one
DMA each — per-tensor loads serialized ~35us on the sync queue).  The heavy
matmul paths run in float32r (TF22), the output MLP in exact fp32.
"""

import os

import numpy as np

import concourse.bacc as bacc
import concourse.bass as bass
import concourse.tile as tile
from concourse import mybir
from concourse.alu_op_type import AluOpType
from concourse.bass_utils import run_bass_kernel_spmd

dt = mybir.dt
AF = mybir.ActivationFunctionType
F32 = dt.float32
F32R = dt.float32r

B, N, NE, DE = 8, 64, 128, 16
MUL = 16
RBF = 32
ZEMB = 32
HID = 128
LAT = 64
RANK = 8
CUTOFF = 6.0
NZ = 101  # MAX_Z + 1
SQ3 = float(np.sqrt(3.0))
C0 = float(1.0 / np.sqrt(2.0 * MUL))
C3 = float(C0 / np.sqrt(3.0))
_DELTA = CUTOFF / (RBF - 1)
GAMMA = float(1.0 / (_DELTA * _DELTA + 1e-12))
PI = float(np.pi)

NB = 8                 # n-values per chunk of the gate loop
NCH = N // NB          # 8 chunks
CW = NB * NE           # 1024 columns per chunk

# True -> run everything in plain fp32 (slower, for accuracy comparison)
FP32_ALL = os.environ.get("FP32_ALL", "0") == "1"
# gate MLP layers 2/3 dtype: f16 (fast weight load) or f32r
GATE16 = os.environ.get("GATE16", "1") == "1" and not FP32_ALL
F16 = dt.float16
# CoreSim does not implement the fused Silu activation; set False to emit
# sigmoid+mul instead (slower, simulation-only)
SILU_FUSED = True
DEBUG = False
_dbg_shapes = {}


def _rdt():
    return F32 if FP32_ALL else F32R


# ---------------------------------------------------------------------------
# pack layouts: name -> (part_rows, col_off, width) within each pack
# ---------------------------------------------------------------------------
PA = {  # f32 [128, *]
    "wb2": (128, 0, 128), "wc2": (128, 128, 128), "we2": (128, 256, 128),
    "wo2": (128, 384, 128), "ident": (128, 512, 128),
    "wc3": (128, 640, 8), "we3": (128, 648, 8), "wo3": (128, 656, 64),
    "bb1": (128, 720, 1), "bb2": (128, 721, 1), "bc1": (128, 722, 1),
    "bc2": (128, 723, 1), "be1": (128, 724, 1), "be2": (128, 725, 1),
    "bo1": (128, 726, 1), "bo2": (128, 727, 1), "bs1": (128, 728, 1),
    "bs2": (128, 729, 1), "bs3": (128, 730, 1), "bc3": (8, 731, 1),
    "be3": (8, 732, 1),
}
PA_W = 736
PR = {  # f32r [128, *]
    "ws2": (128, 0, 128), "wb3": (128, 128, 1024), "ws3sc": (128, 1152, 31),
}
PR_W = 1184
PB = {  # f32 [64, *]
    "wb1": (64, 0, 128), "wc1": (64, 128, 128), "ws1zr": (64, 256, 128),
    "cent": (64, 384, 32),
}
PB_W = 416
PD = {  # f32 [32, *]
    "ws1a": (32, 0, 128), "ws1n": (32, 128, 128), "wo1": (32, 256, 128),
}
PD_W = 384
PE16 = {  # f32 [16, *]
    "eft": (16, 0, 128), "we1": (16, 128, 128), "ws1e": (16, 256, 128),
}
PE16_W = 384
PER = {  # f32r [16, *]
    "rba": (16, 0, 128), "rbb": (16, 128, 128), "rbc": (16, 256, 128),
    "rbd": (16, 384, 128),
}
PER_W = 512
PF = {  # f32 [1, *]
    "bo3": (1, 0, 64), "ones": (1, 64, 128),
}
PF_W = 192
PFR = {  # f32r [1, *]
    "bb3": (1, 0, 1024), "onesr": (1, 1024, 128),
}
PFR_W = 1152
PH = {  # gate dtype [128, *]
    "ws2", "ws3sc",
}
PCORE = {  # f32 [64, *]  (per core)
    "h": (64, 0, 64), "posp": (64, 64, 3), "maskf": (64, 67, 1),
}
PCORE_W = 68

PACKS = [
    ("pA", PA, PA_W, 128, False), ("pR", PR, PR_W, 128, True),
    ("pB", PB, PB_W, 64, False), ("pD", PD, PD_W, 32, False),
    ("pE", PE16, PE16_W, 16, False), ("pEr", PER, PER_W, 16, True),
    ("pF", PF, PF_W, 1, False), ("pFr", PFR, PFR_W, 1, True),
    ("pC", PCORE, PCORE_W, 64, False),
]
PHG = {  # gate weights [128, *] in the gate dtype (f16 or f32r)
    "ws2g": (128, 0, 128), "ws3g": (128, 128, 31),
}
PHG_W = 160


def build_module():
    nc = bacc.Bacc("TRN2", target_bir_lowering=False, debug=False, num_devices=8)

    d = {}
    for pname, layout, width, rows, is_r in PACKS:
        dty = _rdt() if is_r else F32
        d[pname] = nc.dram_tensor(pname, [rows, width], dty,
                                  kind="ExternalInput").ap()
    gdt = F16 if GATE16 else _rdt()
    d["pH"] = nc.dram_tensor("pH", [128, PHG_W], gdt,
                             kind="ExternalInput").ap()
    d["ohz"] = nc.dram_tensor("ohz", [NZ, N], F32, kind="ExternalInput").ap()
    d["zemb"] = nc.dram_tensor("zemb", [NZ, ZEMB], F32,
                               kind="ExternalInput").ap()

    out_d = nc.dram_tensor("out", [NE, LAT], F32, kind="ExternalOutput").ap()

    with tile.TileContext(nc) as tc:
        _emit(nc, tc, d, out_d)

    nc.compile()
    return nc, list(d.keys())


def _emit(nc, tc, d, out_d):
    from contextlib import ExitStack
    ctx = ExitStack()
    with ctx:
        singles = ctx.enter_context(tc.tile_pool(name="singles", bufs=1))
        work = ctx.enter_context(tc.tile_pool(name="work", bufs=2))
        big = ctx.enter_context(tc.tile_pool(name="big", bufs=3))
        pbig = ctx.enter_context(tc.tile_pool(name="pbig", bufs=3, space="PSUM"))
        psm = ctx.enter_context(tc.tile_pool(name="psm", bufs=2, space="PSUM"))

        dma = nc.sync.dma_start
        vec = nc.vector
        act = nc.scalar
        RDT = _rdt()

        def silu(out, in_, bias=0.0):
            if SILU_FUSED:
                act.activation(out=out, in_=in_, func=AF.Silu, bias=bias)
            else:
                z = work.tile(list(out.shape), F32, tag="silu_z")
                act.activation(out=z[:], in_=in_, func=AF.Identity, bias=bias)
                act.activation(out=out, in_=z[:], func=AF.Sigmoid)
                vec.tensor_mul(out, out, z[:])

        def dbg(name, t):
            if DEBUG:
                if t.dtype == F32R:
                    t = t.bitcast(F32)
                elif t.dtype == F16:
                    return
                shape = list(t.shape)
                _dbg_shapes[name] = shape
                dd = nc.dram_tensor("dbg_" + name, shape, F32,
                                    kind="ExternalOutput").ap()
                dma(out=dd, in_=t)

        # ---------- load the packs (one DMA each) ----------
        # per-core + geometry-critical packs first on the sync queue; the
        # big weight packs go on the gpsimd queue so they don't serialize
        # behind each other.
        packs = {}
        W = {}
        first = {"pC", "pB", "pD", "pE"}
        order = sorted(PACKS, key=lambda x: (x[0] not in first,))
        for pname, layout, width, rows, is_r in order:
            ap = d[pname]
            t = singles.tile([rows, width], ap.dtype, tag=pname)
            if pname in first:
                dma(out=t[:], in_=ap)
            else:
                nc.gpsimd.dma_start(out=t[:], in_=ap)
            packs[pname] = t
            for wname, (r, off, wdt) in layout.items():
                W[wname] = t[0:r, off:off + wdt]
        pH = singles.tile([128, PHG_W], d["pH"].dtype, tag="pH")
        dma(out=pH[:], in_=d["pH"])
        for wname, (r_, off_, wdt_) in PHG.items():
            W[wname] = pH[0:r_, off_:off_ + wdt_]
        ohz = singles.tile([NZ, N], F32, tag="ohz")
        dma(out=ohz[:], in_=d["ohz"])
        zemb = singles.tile([NZ, ZEMB], F32, tag="zemb")
        dma(out=zemb[:], in_=d["zemb"])

        h = W["h"]
        posp = W["posp"]
        maskf = W["maskf"]
        cent = W["cent"]
        ident = W["ident"]
        eft = W["eft"]

        # broadcast absorber position to all rows (DMA from DRAM, 0-stride)
        pos0 = singles.tile([N, 3], F32, tag="pos0")
        _, poff, _ = PCORE["posp"]
        dma(out=pos0[:], in_=bass.AP(
            tensor=d["pC"].tensor, offset=poff,
            ap=[[0, N], [1, 3]]))

        # small bias constants (const_aps only pre-registers 0.0 / 1.0)
        eps8 = singles.tile([128, 1], F32, tag="eps8")
        vec.memset(eps8[:], 1e-8)
        halfpi = singles.tile([N, 1], F32, tag="halfpi")
        vec.memset(halfpi[:], PI / 2.0)

        # ---------- stage A: geometry (N on partitions) ----------
        rel = singles.tile([N, 3], F32, tag="rel")
        vec.tensor_sub(rel[:], posp, pos0[:])
        sq = singles.tile([N, 3], F32, tag="sq")
        vec.tensor_mul(sq[:], rel[:], rel[:])
        r2 = singles.tile([N, 1], F32, tag="r2")
        vec.reduce_sum(out=r2[:], in_=sq[:], axis=mybir.AxisListType.X)
        r = singles.tile([N, 1], F32, tag="r")
        act.activation(out=r[:], in_=r2[:], func=AF.Sqrt)

        # invariants sqrt right after r sqrt (shared ACT table)
        hv = h[:, MUL:64].rearrange("p (u i) -> p u i", i=3)
        vsq = singles.tile([N, 48], F32, tag="vsq")
        vec.tensor_mul(vsq.rearrange("p (u i) -> p u i", i=3), hv, hv)
        msum = singles.tile([N, MUL], F32, tag="msum")
        vec.reduce_sum(out=msum[:], in_=vsq.rearrange("p (u i) -> p u i", i=3),
                       axis=mybir.AxisListType.X)
        inv = singles.tile([N, 32], F32, tag="inv")
        vec.tensor_copy(out=inv[:, 0:MUL], in_=h[:, 0:MUL])
        act.activation(out=inv[:, MUL:32], in_=msum[:], func=AF.Sqrt,
                       scale=1.0 / 3.0, bias=eps8[:N])

        rc = singles.tile([N, 1], F32, tag="rc")
        vec.tensor_scalar_min(out=rc[:], in0=r[:], scalar1=CUTOFF)
        rmx = singles.tile([N, 1], F32, tag="rmx")
        vec.tensor_scalar_max(out=rmx[:], in0=r[:], scalar1=1e-8)
        rinv = singles.tile([N, 1], F32, tag="rinv")
        vec.reciprocal(out=rinv[:], in_=rmx[:])

        # sh1 = sqrt(3) * u  (pos columns pre-permuted on host)
        sh1 = singles.tile([N, 3], F32, tag="sh1")
        vec.tensor_scalar(out=sh1[:], in0=rel[:], scalar1=rinv[:, 0:1],
                          scalar2=SQ3, op0=AluOpType.mult, op1=AluOpType.mult)

        # RBF: rr = exp(-gamma * (cent - rc)^2)
        dgs = singles.tile([N, RBF], F32, tag="dgs")
        vec.tensor_scalar_sub(out=dgs[:], in0=cent, scalar1=rc[:, 0:1])
        vec.tensor_mul(dgs[:], dgs[:], dgs[:])
        rr = singles.tile([N, RBF], F32, tag="rr")
        act.activation(out=rr[:], in_=dgs[:], func=AF.Exp, scale=-GAMMA)

        # cwv = 0.5*(cos(pi*rc/6)+1) * (r<=6) * mask, absorber row zeroed
        # cos(x) = sin(pi/2 - x), x = pi*rc/6 in [0, pi]
        cosr = singles.tile([N, 1], F32, tag="cosr")
        act.activation(out=cosr[:], in_=rc[:], func=AF.Sin,
                       scale=-PI / CUTOFF, bias=halfpi[:])
        vcut = singles.tile([N, 1], F32, tag="vcut")
        vec.tensor_scalar(out=vcut[:], in0=r[:], scalar1=CUTOFF, scalar2=None,
                          op0=AluOpType.is_le)
        cwv = singles.tile([N, 1], F32, tag="cwv")
        vec.tensor_scalar(out=cwv[:], in0=cosr[:], scalar1=0.5, scalar2=0.5,
                          op0=AluOpType.mult, op1=AluOpType.add)
        vec.tensor_mul(cwv[:], cwv[:], vcut[:])
        vec.tensor_mul(cwv[:], cwv[:], maskf)
        vec.memset(cwv[0:1, :], 0.0)
        dbg('cwv', cwv[:])

        # vs1[n,u] = sum_i v1[n,u,i]*sh1[n,i]
        vtmp = singles.tile([N, 48], F32, tag="vtmp")
        sh1b = sh1.unsqueeze(1).broadcast_to([N, MUL, 3])
        vec.tensor_mul(vtmp.rearrange("p (u i) -> p u i", i=3), hv, sh1b)
        vs1 = singles.tile([N, MUL], F32, tag="vs1")
        vec.reduce_sum(out=vs1[:], in_=vtmp.rearrange("p (u i) -> p u i", i=3),
                       axis=mybir.AxisListType.X)
        dbg('vs1', vs1[:])
        dbg('sh1', sh1[:])
        dbg('inv', inv[:])

        # ---------- transposes (feat on partitions) ----------
        def transpose_to(src_ap, p_out, f_out, tag, dty=F32):
            ps = psm.tile([p_out, f_out], F32, tag="ps")
            nc.tensor.transpose(out=ps[:], in_=src_ap,
                                identity=ident[:f_out, :f_out])
            t = singles.tile([p_out, f_out], dty, tag=tag)
            vec.tensor_copy(out=t[:], in_=ps[:])
            return t

        invT = transpose_to(inv[:], 32, N, "invT")
        s1T = transpose_to(h[:, 0:MUL], MUL, N, "s1T", RDT)
        vs1T = transpose_to(vs1[:], MUL, N, "vs1T", RDT)
        vT = [transpose_to(h[:, MUL + i:64:3], MUL, N, f"vT{i}", RDT)
              for i in range(3)]

        # zr in n-major layout via one-hot matmul: ohz.T @ zemb -> [N, 32]
        ps_zr = psm.tile([N, 32], F32, tag="ps")
        nc.tensor.matmul(ps_zr[:], ohz[:], zemb[:], start=True, stop=True)
        # geom_nm = [zr | rr] (n on partitions), then one transpose -> [64, N]
        geom_nm = singles.tile([N, 64], F32, tag="geom_nm")
        vec.tensor_copy(out=geom_nm[:, 0:32], in_=ps_zr[:])
        vec.tensor_copy(out=geom_nm[:, 32:64], in_=rr[:])
        geomT = transpose_to(geom_nm[:], 64, N, "geomT")
        dbg('geom_nm', geom_nm[:])

        # ---------- stage B: small MLPs ----------
        def mlp2(w1, b1, w2, b2, rhs, cols, tag, out_dt=F32):
            ps = psm.tile([HID, cols], F32, tag="ps")
            nc.tensor.matmul(ps[:], w1, rhs, start=True, stop=True)
            h1 = work.tile([HID, cols], F32, tag="h1_" + tag)
            silu(h1[:], ps[:], bias=b1)
            ps2 = psm.tile([HID, cols], F32, tag="ps")
            nc.tensor.matmul(ps2[:], w2, h1[:], start=True, stop=True)
            h2 = singles.tile([HID, cols], out_dt, tag="h2_" + tag)
            silu(h2[:], ps2[:], bias=b2)
            return h2

        # base_w trunk on geom_in = [zr, rr]
        h2b = mlp2(W["wb1"], W["bb1"], W["wb2"], W["bb2"], geomT[:], N, "b",
                   out_dt=RDT)

        # base_w[n, :] = h2b.T @ wb3 + bb3   (n on partitions; bias via K=1)
        bw = singles.tile([N, 1024], F32, tag="bw")
        for j in range(2):
            ps = psm.tile([N, 512], F32, tag="ps")
            nc.tensor.matmul(ps[:], h2b[:], W["wb3"][:, 512 * j:512 * (j + 1)],
                             start=True, stop=False)
            nc.tensor.matmul(ps[:], W["onesr"][:, 0:N],
                             W["bb3"][:, 512 * j:512 * (j + 1)],
                             start=False, stop=True)
            vec.tensor_copy(out=bw[:, 512 * j:512 * (j + 1)], in_=ps[:])

        # geom_c
        h2c = mlp2(W["wc1"], W["bc1"], W["wc2"], W["bc2"], geomT[:], N, "c")
        ps_gc = psm.tile([RANK, N], F32, tag="ps")
        nc.tensor.matmul(ps_gc[:], W["wc3"], h2c[:], start=True, stop=True)
        gcT = singles.tile([RANK, N], F32, tag="gcT")
        vec.tensor_scalar_add(out=gcT[:], in0=ps_gc[:], scalar1=W["bc3"])
        gc = transpose_to(gcT[:], N, RANK, "gc")
        dbg('bw', bw[:])
        dbg('gc', gc[:])

        # energy_c (E rows)
        h2e = mlp2(W["we1"], W["be1"], W["we2"], W["be2"], eft, NE, "e")
        ps_ec = psm.tile([RANK, NE], F32, tag="ps")
        nc.tensor.matmul(ps_ec[:], W["we3"], h2e[:], start=True, stop=True)
        ecT = singles.tile([RANK, NE], F32, tag="ecT")
        vec.tensor_scalar_add(out=ecT[:], in0=ps_ec[:], scalar1=W["be3"])
        ec = transpose_to(ecT[:], NE, RANK, "ec")
        dbg('ec', ec[:])

        # ---------- gate layer-1 factored parts ----------
        # xe[k, e] = ws1e.T @ e_feat.T + (bs1 + ws1a.T @ inv_abs)  broadcast
        ps_b1 = psm.tile([HID, 1], F32, tag="ps")
        nc.tensor.matmul(ps_b1[:], W["ws1a"], invT[:, 0:1], start=True,
                         stop=True)
        b1eff = singles.tile([HID, 1], F32, tag="b1eff")
        vec.tensor_add(b1eff[:], ps_b1[:], W["bs1"])
        ps_xe = psm.tile([HID, NE], F32, tag="ps")
        nc.tensor.matmul(ps_xe[:], W["ws1e"], eft, start=True, stop=True)
        xe = singles.tile([HID, NE], F32, tag="xe")
        vec.tensor_scalar_add(out=xe[:], in0=ps_xe[:], scalar1=b1eff[:, 0:1])
        dbg('xeT', xe[:])

        # xn[k, n] = ws1n.T@inv + [ws1z;ws1r].T@[zr;rr]
        ps_xn = psm.tile([HID, N], F32, tag="ps")
        nc.tensor.matmul(ps_xn[:], W["ws1n"], invT[:], start=True, stop=False)
        nc.tensor.matmul(ps_xn[:], W["ws1zr"], geomT[:], start=False, stop=True)
        xn = singles.tile([HID, N], F32, tag="xn")
        vec.tensor_copy(out=xn[:], in_=ps_xn[:])
        dbg('xnT', xn[:])

        # ---------- stage D: VB / VR (values factorization) ----------
        # VRB[n, 0:512]   = VR  (r outer, d inner; includes geom_c factor)
        # VRB[n, 512:576] = VB
        # VRB[n, 576]     = 1.0  (gate-sum column)
        VRB = singles.tile([N, 578], RDT, tag="VRB")
        VR = VRB[:, 0:512].rearrange("p (r e) -> p r e", r=RANK)
        VB = VRB[:, 512:576]
        act.activation(out=VRB[:, 576:578],
                       in_=eps8[:N].broadcast_to([N, 2]), func=AF.Copy,
                       bias=1.0, scale=0.0)

        # --- base paths from bw (layout: path p, u, w at 256p+16u+w) ---
        sv1 = singles.tile([N, 2 * MUL], F32, tag="sv1")  # [s1 | vs1]
        vec.tensor_copy(out=sv1[:, 0:MUL], in_=h[:, 0:MUL])
        vec.tensor_copy(out=sv1[:, MUL:2 * MUL], in_=vs1[:])
        tAD = singles.tile([N, 512], F32, tag="tAD")
        bwAD = bass.AP(tensor=bw.tensor, offset=bw.offset,
                       ap=[list(bw.ap[0]), [768, 2], [16, MUL], [1, MUL]])
        sv1b = sv1.rearrange("p (g u) -> p g u", g=2).unsqueeze(3) \
                  .broadcast_to([N, 2, MUL, MUL])
        vec.tensor_mul(tAD.rearrange("p (g u w) -> p g u w", g=2, u=MUL),
                       bwAD, sv1b)
        sAD = singles.tile([N, 2 * MUL], F32, tag="sAD")
        vec.reduce_sum(out=sAD[:],
                       in_=tAD.rearrange("p (g u w) -> p g w u", g=2, u=MUL),
                       axis=mybir.AxisListType.X)
        vec.tensor_add(VB[:, 0:MUL], sAD[:, 0:MUL], sAD[:, MUL:2 * MUL])

        # B path: Bs[n,w] = sum_u bwB[u,w]*s1[u]
        tB = singles.tile([N, 256], F32, tag="tB")
        s1b = h[:, 0:MUL].unsqueeze(2).broadcast_to([N, MUL, MUL])
        vec.tensor_mul(tB.rearrange("p (u w) -> p u w", u=MUL),
                       bw[:, 256:512].rearrange("p (u w) -> p u w", u=MUL), s1b)
        Bs = singles.tile([N, MUL], F32, tag="Bs")
        vec.reduce_sum(out=Bs[:], in_=tB.rearrange("p (u w) -> p w u", u=MUL),
                       axis=mybir.AxisListType.X)

        # C path: Cv[n,w,i] = sum_u bwC[u,w]*v1[n,u,i]
        tC = singles.tile([N, 768], F32, tag="tC")
        bwC = bw[:, 512:768].rearrange("p (u w) -> p u w", u=MUL).unsqueeze(3) \
                            .broadcast_to([N, MUL, MUL, 3])
        hvb = h[:, MUL:64].rearrange("p (u i) -> p u i", i=3).unsqueeze(2) \
                          .broadcast_to([N, MUL, MUL, 3])
        vec.tensor_mul(tC.rearrange("p (u w i) -> p u w i", u=MUL, w=MUL),
                       bwC, hvb)
        Cv = singles.tile([N, 48], F32, tag="Cv")
        vec.reduce_sum(out=Cv[:],
                       in_=tC.rearrange("p (u w i) -> p w i u", u=MUL, w=MUL),
                       axis=mybir.AxisListType.X)

        # VB vector part: Bs[w]*sh1[i] + Cv[w,i]
        Bsb = Bs.unsqueeze(2).broadcast_to([N, MUL, 3])
        sh1b2 = sh1.unsqueeze(1).broadcast_to([N, MUL, 3])
        tBv = singles.tile([N, 48], F32, tag="tBv")
        vec.tensor_mul(tBv.rearrange("p (w i) -> p w i", i=3), Bsb, sh1b2)
        vec.tensor_add(VB[:, MUL:64], tBv[:], Cv[:])

        # --- rank paths (matmuls, K=16, f32r) ---
        ps_r1 = psm.tile([N, 384], F32, tag="ps")  # [sA | sB | sD]
        nc.tensor.matmul(ps_r1[:, 0:128], s1T[:], W["rba"], start=True,
                         stop=True)
        nc.tensor.matmul(ps_r1[:, 128:256], s1T[:], W["rbb"], start=True,
                         stop=True)
        nc.tensor.matmul(ps_r1[:, 256:384], vs1T[:], W["rbd"], start=True,
                         stop=True)
        ps_r2 = psm.tile([N, 384], F32, tag="ps")  # [PC0 | PC1 | PC2]
        for i in range(3):
            nc.tensor.matmul(ps_r2[:, 128 * i:128 * (i + 1)], vT[i][:],
                             W["rbc"], start=True, stop=True)

        # move to SBUF (DVE can read only one PSUM input per op)
        r1 = singles.tile([N, 384], F32, tag="r1")
        vec.tensor_copy(out=r1[:], in_=ps_r1[:])
        # VR scalar part: gc[r] * (sA + sD)   (layout [r, w] at 16r+w)
        tS = singles.tile([N, 128], F32, tag="tS")
        vec.tensor_add(tS[:], r1[:, 0:128], r1[:, 256:384])
        gcb = gc.unsqueeze(2).broadcast_to([N, RANK, MUL])
        vec.tensor_mul(VR[:, :, 0:MUL].rearrange("p r w -> p r w"),
                       tS.rearrange("p (r w) -> p r w", r=RANK), gcb)

        # VR vector part: gc[r] * (sB[r,w]*sh1[i] + PC_i[r,w])
        for i in range(3):
            tV = singles.tile([N, 128], F32, tag="tV")
            nc.vector.scalar_tensor_tensor(
                out=tV[:], in0=r1[:, 128:256], scalar=sh1[:, i:i + 1],
                in1=ps_r2[:, 128 * i:128 * (i + 1)],
                op0=AluOpType.mult, op1=AluOpType.add)
            vec.tensor_mul(
                VR[:, :, MUL + i:64:3].rearrange("p r w -> p r w"),
                tV.rearrange("p (r w) -> p r w", r=RANK), gcb)

        dbg('VRB', VRB[:])

        # ---------- stage C: gate MLP over E*N rows (n-outer chunks) ----------
        # scores accumulate into [8, 512] psum tiles: the matmul for score
        # half-chunk s uses a shifted zero-padded ws3 (col s%8 = ws3) so its
        # 512 scores land in psum row s%8, zeros elsewhere.
        g_nm = singles.tile([N, NE], RDT, tag="g_nm")  # sigmoid(score)*cwv
        ps_sc = psm.tile([16, 512], F32, tag="ps")
        for c in range(NCH):
            h1p = big.tile([HID, CW], F32, tag="h1p")
            xeb = xe.unsqueeze(1).broadcast_to([HID, NB, NE])
            xnb = xn[:, NB * c:NB * (c + 1)].unsqueeze(2) \
                    .broadcast_to([HID, NB, NE])
            vec.tensor_tensor(h1p.rearrange("p (n e) -> p n e", n=NB), xeb,
                              xnb, op=AluOpType.add)
            h1 = big.tile([HID, CW], F16 if GATE16 else RDT, tag="h1")
            silu(h1[:], h1p[:])
            if c == 0:
                dbg('h1c0', h1[:])
            z2 = pbig.tile([HID, CW], F32, tag="z2")
            for j in range(2):
                sl = slice(512 * j, 512 * (j + 1))
                nc.tensor.matmul(z2[:, sl], W["ws2g"], h1[:, sl],
                                 start=True, stop=True)
            h2 = big.tile([HID, CW], F16 if GATE16 else RDT, tag="h2")
            silu(h2[:], z2[:], bias=W["bs2"])
            if c == 0:
                dbg('h2c0', h2[:])
            for j in range(2):
                s = 2 * c + j          # score half-chunk: n in [4s, 4s+4)
                sl = slice(512 * j, 512 * (j + 1))
                nc.tensor.matmul(ps_sc[:], W["ws3g"][:, 15 - s:31 - s],
                                 h2[:, sl], start=(s == 0), stop=(s == 15))
        # one sigmoid + one reshape DMA for all scores
        sig = work.tile([16, 512], RDT, tag="sig")
        act.activation(out=sig[:], in_=ps_sc[:], func=AF.Sigmoid,
                       bias=W["bs3"][:16])
        dbg('sig0', sig[:])
        dma(out=g_nm[:, :], in_=sig[:].rearrange("p (n e) -> p n e", n=4))

        # gate = sigmoid(score) * cwv
        vec.tensor_scalar_mul(out=g_nm[:], in0=g_nm[:].bitcast(F32),
                              scalar1=cwv[:, 0:1])
        dbg('g_nm', g_nm[:])

        # ---------- aggregation: G = g_nm.T @ VRB ----------
        ps_G = pbig.tile([NE, 578], F32, tag="z2")
        for sl in [slice(0, 512), slice(512, 578)]:
            nc.tensor.matmul(ps_G[:, sl], g_nm[:, :], VRB[:, sl],
                             start=True, stop=True)

        # agg = (G1 + sum_r ec[e,r]*G2[e,r,:]) / max(gsum, 1e-8)
        gs = singles.tile([NE, 1], F32, tag="gs")
        vec.tensor_scalar_max(out=gs[:], in0=ps_G[:, 576:577], scalar1=1e-8)
        gsi = singles.tile([NE, 1], F32, tag="gsi")
        vec.reciprocal(out=gsi[:], in_=gs[:])

        G2v = ps_G[:, 0:512].rearrange("p (r e) -> p e r", r=RANK)
        ecb = ec.unsqueeze(1).broadcast_to([NE, 64, RANK])
        tG = singles.tile([NE, 512], F32, tag="tG")
        vec.tensor_mul(tG.rearrange("p (e r) -> p e r", r=RANK), G2v, ecb)
        aggR = singles.tile([NE, 64], F32, tag="aggR")
        vec.reduce_sum(out=aggR[:],
                       in_=tG.rearrange("p (e r) -> p e r", r=RANK),
                       axis=mybir.AxisListType.X)
        agg = singles.tile([NE, 64], F32, tag="agg")
        vec.tensor_add(agg[:], aggR[:], ps_G[:, 512:576])
        vec.tensor_scalar_mul(out=agg[:], in0=agg[:], scalar1=gsi[:, 0:1])
        dbg('agg', agg[:])

        # ---------- invariants(agg) + out MLP (exact fp32) ----------
        av = agg[:, MUL:64].rearrange("p (u i) -> p u i", i=3)
        asq = singles.tile([NE, 48], F32, tag="asq")
        vec.tensor_mul(asq.rearrange("p (u i) -> p u i", i=3), av, av)
        ams = singles.tile([NE, MUL], F32, tag="ams")
        vec.reduce_sum(out=ams[:],
                       in_=asq.rearrange("p (u i) -> p u i", i=3),
                       axis=mybir.AxisListType.X)
        ia = singles.tile([NE, 32], F32, tag="ia")
        vec.tensor_copy(out=ia[:, 0:MUL], in_=agg[:, 0:MUL])
        act.activation(out=ia[:, MUL:32], in_=ams[:], func=AF.Sqrt,
                       scale=1.0 / 3.0, bias=eps8[:])
        iaT = transpose_to(ia[:], 32, NE, "iaT")

        h2o = mlp2(W["wo1"], W["bo1"], W["wo2"], W["bo2"], iaT[:], NE, "o")

        ps_out = psm.tile([NE, LAT], F32, tag="ps")
        nc.tensor.matmul(ps_out[:], h2o[:], W["wo3"], start=True, stop=False)
        nc.tensor.matmul(ps_out[:], W["ones"][:, 0:NE], W["bo3"],
                         start=False, stop=True)
        osb = singles.tile([NE, LAT], F32, tag="osb")
        vec.tensor_copy(out=osb[:], in_=ps_out[:])
        dma(out=out_d, in_=osb[:])


# ---------------------------------------------------------------------------
# host side
# ---------------------------------------------------------------------------
_CACHED = {}


def _round_fp22(x):
    """round fp32 to nearest-even at 13 mantissa bits (FP22 / fp32r)"""
    if FP32_ALL:
        return np.asarray(x, np.float32)
    u = np.ascontiguousarray(np.asarray(x, np.float32)).view(np.uint32)
    lsb = (u >> 13) & 1
    u = (u + 0x0FFF + lsb) & np.uint32(0xFFFFE000)
    return u.view(np.float32)


def _prep_maps(h_full, z, pos, mask, e_feat, absorber_index, z_emb,
               base_w_mlp, geom_c_mlp, energy_c_mlp, score_mlp, out_mlp,
               rank_basis):
    f = np.float32
    a = int(absorber_index)
    h_full = np.asarray(h_full, f)
    z = np.asarray(z)
    pos = np.asarray(pos, f)
    mask = np.asarray(mask)
    if a != 0:
        h_full = np.roll(h_full, -a, axis=1)
        z = np.roll(z, -a, axis=1)
        pos = np.roll(pos, -a, axis=1)
        mask = np.roll(mask, -a, axis=1)

    e_feat = np.asarray(e_feat, f)
    z_emb = np.asarray(z_emb, f)
    rank_basis = np.asarray(rank_basis, f)

    def p(t):
        return np.ascontiguousarray(np.asarray(t, f))

    vals = {}
    (Wb1, bb1), (Wb2, bb2), (Wb3, bb3) = base_w_mlp
    scale = np.concatenate([np.full(256, s, f) for s in (C0, C0, C0, C3)])
    vals["wb1"] = p(Wb1); vals["bb1"] = p(bb1)[:, None]
    vals["wb2"] = p(Wb2); vals["bb2"] = p(bb2)[:, None]
    vals["wb3"] = p(Wb3) * scale
    vals["bb3"] = (p(bb3) * scale)[None, :]

    (Wc1, bc1), (Wc2, bc2), (Wc3, bc3) = geom_c_mlp
    vals["wc1"] = p(Wc1); vals["bc1"] = p(bc1)[:, None]
    vals["wc2"] = p(Wc2); vals["bc2"] = p(bc2)[:, None]
    vals["wc3"] = p(Wc3); vals["bc3"] = p(bc3)[:, None]

    (We1, be1), (We2, be2), (We3, be3) = energy_c_mlp
    vals["we1"] = p(We1); vals["be1"] = p(be1)[:, None]
    vals["we2"] = p(We2); vals["be2"] = p(be2)[:, None]
    vals["we3"] = p(We3); vals["be3"] = p(be3)[:, None]

    (Ws1, bs1), (Ws2, bs2), (Ws3, bs3) = score_mlp
    Ws1 = p(Ws1)
    vals["ws1a"] = Ws1[0:32]
    vals["ws1n"] = Ws1[32:64]
    vals["ws1zr"] = Ws1[64:128]
    vals["ws1e"] = Ws1[128:144]
    vals["bs1"] = p(bs1)[:, None]
    vals["ws2"] = p(Ws2)
    vals["bs2"] = p(bs2)[:, None]
    w3p = np.zeros((HID, 31), f)
    w3p[:, 15] = p(Ws3)[:, 0]
    vals["ws3sc"] = w3p
    vals["bs3"] = np.full((128, 1), float(np.asarray(bs3, f)[0]), f)

    (Wo1, bo1), (Wo2, bo2), (Wo3, bo3) = out_mlp
    vals["wo1"] = p(Wo1); vals["bo1"] = p(bo1)[:, None]
    vals["wo2"] = p(Wo2); vals["bo2"] = p(bo2)[:, None]
    vals["wo3"] = p(Wo3); vals["bo3"] = p(bo3)[None, :]

    rb = rank_basis.reshape(RANK, 4, MUL, MUL)
    for name, pi, s in [("rba", 0, C0), ("rbb", 1, C0), ("rbc", 2, C0),
                        ("rbd", 3, C3)]:
        vals[name] = np.ascontiguousarray(
            (rb[:, pi] * s).transpose(1, 0, 2).reshape(MUL, RANK * MUL))

    centers = np.linspace(0.0, CUTOFF, RBF, dtype=f)
    vals["cent"] = np.ascontiguousarray(np.broadcast_to(centers, (N, RBF)))
    vals["ident"] = np.eye(128, dtype=f)
    vals["ones"] = np.ones((1, 128), f)
    vals["onesr"] = np.ones((1, 128), f)
    vals["eft"] = p(e_feat.T)

    def build_pack(layout, width, rows, is_r):
        buf = np.zeros((rows, width), f)
        for wname, (r, off, wdt) in layout.items():
            if wname in ("h", "posp", "maskf"):
                continue
            v = vals[wname]
            assert v.shape == (r, wdt), (wname, v.shape, (r, wdt))
            buf[0:r, off:off + wdt] = v
        return _round_fp22(buf) if is_r else buf

    shared = {}
    for pname, layout, width, rows, is_r in PACKS:
        if pname == "pC":
            continue
        shared[pname] = build_pack(layout, width, rows, is_r)
    ph = np.zeros((128, PHG_W), np.float32)
    ph[:, 0:128] = vals["ws2"]
    ph[:, 128:159] = vals["ws3sc"]
    shared["pH"] = ph.astype(np.float16) if GATE16 else _round_fp22(ph)
    shared["zemb"] = p(z_emb)

    maps = []
    for b in range(B):
        m = dict(shared)
        pc = np.zeros((64, PCORE_W), f)
        pc[:, 0:64] = h_full[b]
        pc[:, 64:67] = pos[b][:, [1, 2, 0]]
        pc[:, 67:68] = np.asarray(mask[b], f)[:, None]
        m["pC"] = pc
        oh = np.zeros((NZ, N), f)
        oh[np.asarray(z[b], np.int64), np.arange(N)] = 1.0
        m["ohz"] = oh
        maps.append(m)
    return maps


def kernel(**inputs) -> np.ndarray:
    if "nc" not in _CACHED:
        nc, in_names = build_module()
        _CACHED["nc"] = nc
        _CACHED["in_names"] = in_names
    nc = _CACHED["nc"]
    maps = _prep_maps(**inputs)
    res = run_bass_kernel_spmd(nc, maps, list(range(B)))
    return np.stack([res.results[b]["out"] for b in range(B)], axis=0)


if __name__ == "__main__":
    nc, names = build_module()
    print("built ok", len(names), "inputs")
